# revision 16
# baseline (speedup 1.0000x reference)
"""GCN layer kernel for Trainium2, 8 NeuronCores (SPMD, one NEFF).

Computation (reference):
    h   = features @ W.T + b
    msg = h[src] * norm[src]
    agg = segment_sum(msg, dst)
    out = relu(agg * norm)

Measured reality of this environment: GpSimd indexed-DMA calls are the
binding resource (~8us per 1024-idx call: SWDGE descriptor generation
plus a per-queue descriptor-ring-limited DMA drain); calls are capped
at num_idxs<=1024 (gather) / 4096 (scatter-add) — bigger gathers hang
the mesh regardless of the ring carveout.  The whole message pipeline
runs in fp16 (rel err 1.7e-3, budget 2e-2), halving edge and
collective bytes.  The design minimizes indexed-DMA call count,
spreads calls over all 4 SWDGE queues (per-queue rings drain
concurrently; 5-deep edge-tile buffering keeps them fed; each block's
scatter is split 2x2048 onto two queues — 2-way was measured faster
than 1- and 4-way), and overlaps everything else underneath:

- dst nodes sharded by id (12544/core).  Linear phase is sharded: each
  core computes g rows for its own shard only (norm[src] is folded on
  the host: X' = X * norm, so g = X'@W.T + norm x b; the rank-1 bias
  term is two K=1 matmul instructions per 128-node tile), then one
  AllGather collective replicates the full g table to every core.
- edge phase: per-edge dma_gather (4 x 1024-idx calls fill a 4096-slot
  block) + one dma_scatter_add(4096) per block into a DRAM accumulator.
  dma_scatter_add races on duplicate indices within a call, so host
  packing assigns edges of one dst to different blocks (sorted position
  i -> chunk i % 14; per-(dst, src-super-shard) degree <= 14 guaranteed
  by assignment, rare excess edges park their message in a scratch row
  and are replayed by small fixup gather+scatter calls).  Padding slots
  gather row 0 and scatter into a trash block.
- epilogue: out = norm * relu(agg) (norm > 0 commutes with relu), as
  batched 1792-wide ACT relu + in-place DVE multiply with a
  stride-0-broadcast norm operand, 4 instructions per 14 tiles.
- overlap: the table AllGather is split into 4 quarter-slice
  collectives, each issued as soon as its linear-phase rows are
  written; the edge phase walks blocks super-major so super-q gathers
  start right after collective q lands while later slices transfer.
  The 4 gathers of a block go to SWDGE queues 0-3; the two scatter
  halves rotate over queue pairs.

Gather indices are int16, so the g table is addressed as 4 super-shards
of 25088 rows (quarter-slice AllGather layout: super q holds rows
c*3136..c*3136+3135 of every core c); each 4096-block holds edges of a
single super-shard.
"""
import numpy as np

from concourse import bacc, bass
import concourse.mybir as mybir
from concourse.tile import TileContext
from concourse.bass_utils import run_bass_kernel_spmd

# ---------------------------------------------------------------- config
N_NODES = 100000
D = 128
N_CORES = 8
DSHARD = 12544                 # dst rows per core (98 tiles)
NPAD = 100352                  # g-table rows (784 tiles); 8*DSHARD
NSUPER = 4
SUPER = NPAD // NSUPER         # 25088 rows per gather super-shard (< 2^15)
N_CHUNKS = 14                  # scatter blocks per super-shard stream
BLK = 4096                     # slots per scatter block
GB = 1024                      # gather call size (HW limit)
DMA_SCRATCH = 16384            # desc ring (default)
NBLK = N_CHUNKS * NSUPER       # 56 blocks per core
AGG_ROWS = DSHARD + 256        # + trash block (128) + scratch block (128)
TRASH = DSHARD
SCRATCH0 = DSHARD + 128
NFIX = 4                       # fixup blocks (dataset excess <= 5/core)
FIXBLK = 128                   # slots per fixup block
LIN_MACRO = 7                  # node-tiles per linear macro (98 = 14*7)
FIN_MACRO = 14                 # node-tiles per epilogue macro

_nc_cache = {}
PHASE = "full"   # diagnostics: lin | linag | full


def build_nc(gb4=1024):
    key = (N_NODES, DSHARD, NPAD, N_CHUNKS, BLK, PHASE, gb4)
    if key in _nc_cache:
        return _nc_cache[key]
    nc = bacc.Bacc("TRN2", target_bir_lowering=False, num_devices=N_CORES,
                   dynamic_dma_scratch_size=DMA_SCRATCH, num_swdge_queues=4)
    f32, i16 = mybir.dt.float32, mybir.dt.int16
    f16 = mybir.dt.float16
    ntile_dst = DSHARD // 128          # 98
    icols = NBLK * BLK // 16           # scatter/gather idx cols
    ficols = NFIX * FIXBLK // 16

    fto = nc.dram_tensor("fto", [128, DSHARD], f32, kind="ExternalInput")
    wt = nc.dram_tensor("wt", [128, 128], f32, kind="ExternalInput")
    biasr = nc.dram_tensor("biasr", [1, 128], f32, kind="ExternalInput")
    normr = nc.dram_tensor("normr", [1, DSHARD], f32, kind="ExternalInput")
    normt_dst = nc.dram_tensor("normt_dst", [128, ntile_dst], f32, kind="ExternalInput")
    gidx = nc.dram_tensor("gidx", [128, icols], i16, kind="ExternalInput")
    sidx = nc.dram_tensor("sidx", [128, icols], i16, kind="ExternalInput")
    fgidx = nc.dram_tensor("fgidx", [128, ficols], i16, kind="ExternalInput")
    fsidx = nc.dram_tensor("fsidx", [128, ficols], i16, kind="ExternalInput")
    out = nc.dram_tensor("out", [DSHARD, D], f32, kind="ExternalOutput")

    g_own = nc.dram_tensor("g_own", [DSHARD, D], f16, kind="Internal")
    g_full = nc.dram_tensor("g_full", [NPAD, D], f16, kind="Internal",
                            addr_space="Shared")
    agg = nc.dram_tensor("agg", [AGG_ROWS, D], f16, kind="Internal")

    gov = g_own.reshape([ntile_dst, 128, D])
    aggv = agg.reshape([AGG_ROWS // 128, 128, D])
    outv = out.reshape([ntile_dst, 128, D])

    with TileContext(nc) as tc:
        with (
            tc.tile_pool(name="const", bufs=1) as cpool,
            tc.tile_pool(name="mmin", bufs=2) as mmpool,
            tc.tile_pool(name="psmm", bufs=2, space="PSUM") as psmm,
            tc.tile_pool(name="gout", bufs=2) as gpool,
            tc.tile_pool(name="edge", bufs=5) as epool,
            tc.tile_pool(name="fin", bufs=2) as fpool,
        ):
            wt_sb = cpool.tile([128, 128], f32)
            bias_sb = cpool.tile([1, 128], f32)
            ndst_sb = cpool.tile([128, ntile_dst], f32)
            gi_sb = cpool.tile([128, icols], i16)
            si_sb = cpool.tile([128, icols], i16)
            fgi_sb = cpool.tile([128, ficols], i16)
            fsi_sb = cpool.tile([128, ficols], i16)
            z_sb = cpool.tile([128, 33 * 128], f16)

            nc.sync.dma_start(out=wt_sb[:], in_=wt[:])
            nc.sync.dma_start(out=bias_sb[:], in_=biasr[:])
            nc.sync.dma_start(out=ndst_sb[:], in_=normt_dst[:])
            nc.sync.dma_start(out=gi_sb[:], in_=gidx[:])
            nc.sync.dma_start(out=si_sb[:], in_=sidx[:])
            nc.sync.dma_start(out=fgi_sb[:], in_=fgidx[:])
            nc.sync.dma_start(out=fsi_sb[:], in_=fsidx[:])
            nc.vector.memset(z_sb[:], 0.0)

            # zero the accumulator table (real cfg: 100 tiles = 10 x 10)
            nzt = AGG_ROWS // 128
            k = 0
            while k < nzt:
                gsz = min(33, nzt - k)
                nc.sync.dma_start(
                    out=aggv[k:k + gsz].rearrange("a p d -> p a d"),
                    in_=z_sb[:, :gsz * 128].rearrange("p (a d) -> p a d", a=gsz))
                k += gsz

            # ---- sharded linear: g_own = X'@W.T + norm x b ----
            for m in range(ntile_dst // LIN_MACRO):
                fch = mmpool.tile([128, LIN_MACRO * 128], f32, tag="fch")
                nc.sync.dma_start(
                    out=fch[:], in_=fto[:, m * LIN_MACRO * 128:(m + 1) * LIN_MACRO * 128])
                nrm = mmpool.tile([1, LIN_MACRO * 128], f32, tag="nrm")
                nc.sync.dma_start(
                    out=nrm[:], in_=normr[:, m * LIN_MACRO * 128:(m + 1) * LIN_MACRO * 128])
                gsb = gpool.tile([128, LIN_MACRO, 128], f16, tag="gsb")
                ps = psmm.tile([128, 8, 128], f32, tag="psm")
                for j in range(LIN_MACRO):
                    nc.tensor.matmul(ps[:, j, :], fch[:, j * 128:(j + 1) * 128],
                                     wt_sb[:], start=True, stop=False)
                    nc.tensor.matmul(ps[:, j, :], nrm[:, j * 128:(j + 1) * 128],
                                     bias_sb[:], start=False, stop=True)
                if m % 2 == 0:
                    nc.vector.tensor_copy(
                        gsb[:].rearrange("p a d -> p (a d)"),
                        ps[:, 0:LIN_MACRO, :].rearrange("p a d -> p (a d)"))
                else:
                    nc.scalar.activation(
                        gsb[:].rearrange("p a d -> p (a d)"),
                        ps[:, 0:LIN_MACRO, :].rearrange("p a d -> p (a d)"),
                        mybir.ActivationFunctionType.Copy)
                nc.sync.dma_start(
                    out=gov[m * LIN_MACRO:(m + 1) * LIN_MACRO].rearrange("a p d -> p a d"),
                    in_=gsb[:])
                # chunked table replication: AG slice q as soon as its rows
                # are written (3136 = 3.5 macros; boundaries after macros
                # 3, 6, 10, 13)
                if PHASE != "lin" and m in (3, 6, 10, 13):
                    q = (3, 6, 10, 13).index(m)
                    qs = DSHARD // NSUPER
                    nc.gpsimd.collective_compute(
                        "AllGather", mybir.AluOpType.bypass,
                        [list(range(N_CORES))],
                        [g_own[q * qs:(q + 1) * qs, :]],
                        [g_full[q * SUPER:(q + 1) * SUPER, :]])

            # ---- edge phase ----
            border = [c * NSUPER + s for s in range(NSUPER)
                      for c in range(N_CHUNKS)]
            for bkt in (border if PHASE == "full" else []):
                s = bkt % NSUPER
                tg = epool.tile([128, BLK // 128, D], f16, tag="tg")
                off = 0
                for q, gsz in enumerate((GB, GB, GB, gb4)):
                    c0 = bkt * (BLK // 16) + off // 16
                    nc.gpsimd.dma_gather(
                        tg[:, off // 128:(off + gsz) // 128, :],
                        g_full[s * SUPER:(s + 1) * SUPER, :],
                        gi_sb[:, c0:c0 + gsz // 16], gsz, gsz, D,
                        queue_num=(q + bkt) % 4)
                    off += gsz
                hb = BLK // 2
                sh2 = 3 * GB + gb4 - hb     # real fill beyond half 1
                for h, hsz in enumerate((hb, sh2)):
                    nc.gpsimd.dma_scatter_add(
                        agg[:], tg[:, h * (hb // 128):h * (hb // 128) + hsz // 128, :],
                        si_sb[:, bkt * (BLK // 16) + h * (hb // 16):
                              bkt * (BLK // 16) + h * (hb // 16) + hsz // 16],
                        hsz, hsz, D, queue_num=(bkt + 2 * h + 1) % 4)

            # ---- fixup: replay over-degree edges parked in scratch rows ----
            for fb in (range(NFIX) if PHASE == "full" else []):
                tf = epool.tile([128, FIXBLK // 128, D], f16, tag="tf")
                cols = slice(fb * (FIXBLK // 16), (fb + 1) * (FIXBLK // 16))
                nc.gpsimd.dma_gather(
                    tf[:], agg[:], fgi_sb[:, cols], FIXBLK, FIXBLK, D)
                nc.gpsimd.dma_scatter_add(
                    agg[:], tf[:], fsi_sb[:, cols], FIXBLK, FIXBLK, D)

            # ---- epilogue: out = norm * relu(agg) ----
            for m in (range(ntile_dst // FIN_MACRO) if PHASE == "full" else []):
                asb = fpool.tile([128, FIN_MACRO, 128], f16, tag="asb")
                nc.sync.dma_start(
                    out=asb[:],
                    in_=aggv[m * FIN_MACRO:(m + 1) * FIN_MACRO].rearrange("a p d -> p a d"))
                rsb = fpool.tile([128, FIN_MACRO, 128], f32, tag="rsb")
                nc.scalar.activation(
                    rsb[:].rearrange("p a d -> p (a d)"),
                    asb[:].rearrange("p a d -> p (a d)"),
                    mybir.ActivationFunctionType.Relu)
                nb = bass.AP(ndst_sb.tensor,
                             ndst_sb.offset + m * FIN_MACRO,
                             [[ntile_dst, 128], [1, FIN_MACRO], [0, 128]])
                nc.vector.tensor_tensor(rsb[:], rsb[:], nb, mybir.AluOpType.mult)
                nc.sync.dma_start(
                    out=outv[m * FIN_MACRO:(m + 1) * FIN_MACRO].rearrange("a p d -> p a d"),
                    in_=rsb[:])

    if PHASE != "full":
        # variants still must write the output tensor
        with TileContext(nc) as tc2:
            with tc2.tile_pool(name="dummy", bufs=1) as dpool:
                zz = dpool.tile([128, 10 * 128], f32)
                nc.vector.memset(zz[:], 0.0)
                k = 0
                while k < ntile_dst:
                    gsz = min(10, ntile_dst - k)
                    nc.sync.dma_start(
                        out=outv[k:k + gsz].rearrange("a p d -> p a d"),
                        in_=zz[:, :gsz * 128].rearrange("p (a d) -> p a d", a=gsz))
                    k += gsz
    nc.compile()
    _nc_cache[key] = nc
    return nc


# ---------------------------------------------------------------- host pack
def _wrap16(stream: np.ndarray) -> np.ndarray:
    """idx i at [i%16, i//16], replicated x8 across partition groups."""
    a = stream.astype(np.int16).reshape(-1, 16).T
    return np.tile(a, (8, 1))


def _pack_core_edges(src_c: np.ndarray, dst_l: np.ndarray):
    """Build per-core index streams for the scatter-based edge phase.

    Returns (gidx_stream [NBLK*BLK], sidx_stream [NBLK*BLK],
             fgidx_stream, fsidx_stream)."""
    g_stream = np.zeros(NBLK * BLK, dtype=np.int64)
    s_stream = TRASH + (np.arange(NBLK * BLK) % 128)
    fg_stream = np.zeros(NFIX * FIXBLK, dtype=np.int64)
    fs_stream = TRASH + (np.arange(NFIX * FIXBLK) % 128)

    excess = []  # (dst_local, scratch_row)
    n_scratch = 0

    qc = src_c // DSHARD
    qr = src_c % DSHARD
    sg = qr // (DSHARD // NSUPER)
    sl = qc * (DSHARD // NSUPER) + qr % (DSHARD // NSUPER)
    for s in range(NSUPER):
        m = sg == s
        sls, dls = sl[m], dst_l[m]
        order = np.argsort(dls, kind="stable")
        sls, dls = sls[order], dls[order]
        n = len(dls)
        if n == 0:
            continue
        chunk = np.arange(n) % N_CHUNKS
        first = np.searchsorted(dls, dls, side="left")
        occ = np.arange(n) - first
        ok = occ < N_CHUNKS

        sc_rows = []
        for edl in dls[~ok].tolist():
            assert n_scratch < 128, "scratch overflow"
            excess.append((edl, SCRATCH0 + n_scratch))
            sc_rows.append(SCRATCH0 + n_scratch)
            n_scratch += 1
        dls = dls.copy()
        if sc_rows:
            dls[~ok] = np.asarray(sc_rows, dtype=np.int64)

        corder = np.argsort(chunk, kind="stable")
        sls, dls, chunk = sls[corder], dls[corder], chunk[corder]
        counts = np.bincount(chunk, minlength=N_CHUNKS)
        _pack_core_edges.maxfill = max(
            getattr(_pack_core_edges, "maxfill", 0), int(counts.max()))
        offs = np.concatenate([[0], np.cumsum(counts)])
        for c in range(N_CHUNKS):
            nb = counts[c]
            assert nb <= BLK, f"block overflow {nb} > {BLK}"
            base = (c * NSUPER + s) * BLK
            g_stream[base:base + nb] = sls[offs[c]:offs[c + 1]]
            s_stream[base:base + nb] = dls[offs[c]:offs[c + 1]]

    fill = [0] * NFIX
    fsets = [set() for _ in range(NFIX)]
    for edl, srow in excess:
        for fb in range(NFIX):
            if fill[fb] < FIXBLK and edl not in fsets[fb]:
                fg_stream[fb * FIXBLK + fill[fb]] = srow
                fs_stream[fb * FIXBLK + fill[fb]] = edl
                fsets[fb].add(edl)
                fill[fb] += 1
                break
        else:
            raise RuntimeError("fixup overflow: increase NFIX")
    return g_stream, s_stream, fg_stream, fs_stream


def pack_inputs(features, norm, W, b, src, dst):
    features = np.asarray(features, dtype=np.float32)
    norm = np.asarray(norm, dtype=np.float32).reshape(-1)
    W = np.asarray(W, dtype=np.float32)
    b = np.asarray(b, dtype=np.float32)
    src = np.asarray(src).astype(np.int64)
    dst = np.asarray(dst).astype(np.int64)
    n = features.shape[0]

    xp = features * norm[:, None]          # fold norm[src] into X
    wt = np.ascontiguousarray(W.T)
    biasr = b.reshape(1, 128).astype(np.float32)
    norm_pad = np.zeros(NPAD, dtype=np.float32)
    norm_pad[:n] = norm

    shared = {"wt": wt, "biasr": biasr}

    owner = dst // DSHARD
    in_maps = []
    for c in range(N_CORES):
        m = owner == c
        gs, ss, fgs, fss = _pack_core_edges(src[m], dst[m] - c * DSHARD)
        lo = c * DSHARD
        hi = min(lo + DSHARD, n)
        fto = np.zeros((128, DSHARD), dtype=np.float32)
        if hi > lo:
            fto[:, :hi - lo] = xp[lo:hi].T
        normr = norm_pad[lo:lo + DSHARD].reshape(1, DSHARD)
        normt_dst = np.ascontiguousarray(norm_pad[lo:lo + DSHARD].reshape(-1, 128).T)
        in_maps.append(dict(shared,
                            fto=fto,
                            normr=np.ascontiguousarray(normr),
                            normt_dst=normt_dst,
                            gidx=_wrap16(gs),
                            sidx=_wrap16(ss),
                            fgidx=_wrap16(fgs),
                            fsidx=_wrap16(fss)))
    # trim the 4th gather call of every block to the data's real need
    # (padding slots beyond it scatter stale-but-finite data to trash rows)
    maxfill = _pack_core_edges.maxfill
    gb4 = min(GB, max(128, int(-(-(maxfill + 128 - 3 * GB) // 128)) * 128))
    return in_maps, gb4


def kernel(**inputs) -> np.ndarray:
    in_maps, gb4 = pack_inputs(inputs["features"], inputs["norm"], inputs["W"],
                               inputs["b"], inputs["src"], inputs["dst"])
    nc = build_nc(gb4)
    res = run_bass_kernel_spmd(nc, in_maps, core_ids=list(range(N_CORES)))
    n = np.asarray(inputs["features"]).shape[0]
    out = np.empty((n, D), dtype=np.float32)
    for c in range(N_CORES):
        lo = c * DSHARD
        hi = min(lo + DSHARD, n)
        if hi > lo:
            out[lo:hi] = res.results[c]["out"][:hi - lo]
    return out



# revision 17
# speedup vs baseline: 1.3916x; 1.3916x over previous
"""GCN layer kernel for Trainium2, 8 NeuronCores (SPMD, one NEFF).

Computation (reference):
    h   = features @ W.T + b
    msg = h[src] * norm[src]
    agg = segment_sum(msg, dst)
    out = relu(agg * norm)

Measured reality of this environment: GpSimd indexed-DMA calls are the
binding resource (~8us per 1024-idx call: SWDGE descriptor generation
plus a per-queue descriptor-ring-limited DMA drain); calls are capped
at num_idxs<=1024 (gather) / 4096 (scatter-add) — bigger gathers hang
the mesh regardless of the ring carveout.  The whole message pipeline
runs in fp16 (rel err 1.7e-3, budget 2e-2), halving edge and
collective bytes.  The design minimizes indexed-DMA call count,
spreads calls over all 4 SWDGE queues (per-queue rings drain
concurrently; 5-deep edge-tile buffering keeps them fed; each block's
scatter is split 2x2048 onto two queues — 2-way was measured faster
than 1- and 4-way), and overlaps everything else underneath:

- dst nodes sharded by id (12544/core).  Linear phase is sharded: each
  core computes g rows for its own shard only (norm[src] is folded on
  the host: X' = X * norm, so g = X'@W.T + norm x b; the rank-1 bias
  term is two K=1 matmul instructions per 128-node tile), then one
  AllGather collective replicates the full g table to every core.
- edge phase: per-edge dma_gather (3 x 1024-idx calls + one trimmed to
  the dataset's real fill) + two dma_scatter_add halves (2048 + trimmed
  remainder) per 4096-slot block into a DRAM accumulator.
  dma_scatter_add races on duplicate indices within a call, so host
  packing assigns edges of one dst to different blocks (sorted position
  i -> chunk i % 14; per-(dst, src-super-shard) degree <= 14 guaranteed
  by assignment, rare excess edges park their message in a scratch row
  and are replayed by small fixup gather+scatter calls).  Padding slots
  gather row 0 and scatter into a trash block.
- epilogue: out = norm * relu(agg) (norm > 0 commutes with relu), as
  batched 1792-wide ACT relu + in-place DVE multiply with a
  stride-0-broadcast norm operand, 4 instructions per 14 tiles.
- overlap: the table AllGather is split into 4 quarter-slice
  collectives, each issued as soon as its linear-phase rows are
  written; the edge phase walks blocks super-major so super-q gathers
  start right after collective q lands while later slices transfer.
  The 4 gathers of a block go to SWDGE queues 0-3; the two scatter
  halves rotate over queue pairs.

Gather indices are int16, so the g table is addressed as 4 super-shards
of 25088 rows (quarter-slice AllGather layout: super q holds rows
c*3136..c*3136+3135 of every core c); each 4096-block holds edges of a
single super-shard.
"""
import numpy as np

from concourse import bacc, bass
import concourse.mybir as mybir
from concourse.tile import TileContext
from concourse.bass_utils import run_bass_kernel_spmd

# ---------------------------------------------------------------- config
N_NODES = 100000
D = 128
N_CORES = 8
DSHARD = 12544                 # dst rows per core (98 tiles)
NPAD = 100352                  # g-table rows (784 tiles); 8*DSHARD
NSUPER = 4
SUPER = NPAD // NSUPER         # 25088 rows per gather super-shard (< 2^15)
N_CHUNKS = 14                  # scatter blocks per super-shard stream
BLK = 4096                     # slots per scatter block
GB = 1024                      # gather call size (HW limit)
DMA_SCRATCH = 16384            # desc ring (default)
NBLK = N_CHUNKS * NSUPER       # 56 blocks per core
AGG_ROWS = DSHARD + 256        # + trash block (128) + scratch block (128)
TRASH = DSHARD
SCRATCH0 = DSHARD + 128
NFIX = 4                       # fixup blocks (dataset excess <= 5/core)
FIXBLK = 128                   # slots per fixup block
LIN_MACRO = 7                  # node-tiles per linear macro (98 = 14*7)
FIN_MACRO = 14                 # node-tiles per epilogue macro

_nc_cache = {}
PHASE = "full"   # diagnostics: lin | linag | full


def build_nc(gb4=1024):
    key = (N_NODES, DSHARD, NPAD, N_CHUNKS, BLK, PHASE, gb4)
    if key in _nc_cache:
        return _nc_cache[key]
    nc = bacc.Bacc("TRN2", target_bir_lowering=False, num_devices=N_CORES,
                   dynamic_dma_scratch_size=DMA_SCRATCH, num_swdge_queues=4)
    f32, i16 = mybir.dt.float32, mybir.dt.int16
    f16 = mybir.dt.float16
    ntile_dst = DSHARD // 128          # 98
    icols = NBLK * BLK // 16           # scatter/gather idx cols
    ficols = NFIX * FIXBLK // 16

    fto = nc.dram_tensor("fto", [128, DSHARD], f32, kind="ExternalInput")
    wt = nc.dram_tensor("wt", [128, 128], f32, kind="ExternalInput")
    biasr = nc.dram_tensor("biasr", [1, 128], f32, kind="ExternalInput")
    normr = nc.dram_tensor("normr", [1, DSHARD], f32, kind="ExternalInput")
    normt_dst = nc.dram_tensor("normt_dst", [128, ntile_dst], f32, kind="ExternalInput")
    gidx = nc.dram_tensor("gidx", [128, icols], i16, kind="ExternalInput")
    sidx = nc.dram_tensor("sidx", [128, icols], i16, kind="ExternalInput")
    fgidx = nc.dram_tensor("fgidx", [128, ficols], i16, kind="ExternalInput")
    fsidx = nc.dram_tensor("fsidx", [128, ficols], i16, kind="ExternalInput")
    out = nc.dram_tensor("out", [DSHARD, D], f32, kind="ExternalOutput")

    g_own = nc.dram_tensor("g_own", [DSHARD, D], f16, kind="Internal")
    g_full = nc.dram_tensor("g_full", [NPAD, D], f16, kind="Internal",
                            addr_space="Shared")
    agg = nc.dram_tensor("agg", [AGG_ROWS, D], f16, kind="Internal")

    gov = g_own.reshape([ntile_dst, 128, D])
    aggv = agg.reshape([AGG_ROWS // 128, 128, D])
    outv = out.reshape([ntile_dst, 128, D])

    with TileContext(nc) as tc:
        with (
            tc.tile_pool(name="const", bufs=1) as cpool,
            tc.tile_pool(name="mmin", bufs=2) as mmpool,
            tc.tile_pool(name="psmm", bufs=2, space="PSUM") as psmm,
            tc.tile_pool(name="gout", bufs=2) as gpool,
            tc.tile_pool(name="edge", bufs=5) as epool,
            tc.tile_pool(name="fin", bufs=2) as fpool,
        ):
            wt_sb = cpool.tile([128, 128], f32)
            bias_sb = cpool.tile([1, 128], f32)
            ndst_sb = cpool.tile([128, ntile_dst], f32)
            gi_sb = cpool.tile([128, icols], i16)
            si_sb = cpool.tile([128, icols], i16)
            fgi_sb = cpool.tile([128, ficols], i16)
            fsi_sb = cpool.tile([128, ficols], i16)
            z_sb = cpool.tile([128, 33 * 128], f16)

            nc.sync.dma_start(out=wt_sb[:], in_=wt[:])
            nc.sync.dma_start(out=bias_sb[:], in_=biasr[:])
            nc.sync.dma_start(out=ndst_sb[:], in_=normt_dst[:])
            nc.sync.dma_start(out=gi_sb[:], in_=gidx[:])
            nc.sync.dma_start(out=si_sb[:], in_=sidx[:])
            nc.sync.dma_start(out=fgi_sb[:], in_=fgidx[:])
            nc.sync.dma_start(out=fsi_sb[:], in_=fsidx[:])
            nc.vector.memset(z_sb[:], 0.0)

            # zero the accumulator table (real cfg: 100 tiles = 10 x 10)
            nzt = AGG_ROWS // 128
            k = 0
            while k < nzt:
                gsz = min(33, nzt - k)
                nc.sync.dma_start(
                    out=aggv[k:k + gsz].rearrange("a p d -> p a d"),
                    in_=z_sb[:, :gsz * 128].rearrange("p (a d) -> p a d", a=gsz))
                k += gsz

            # ---- sharded linear: g_own = X'@W.T + norm x b ----
            for m in range(ntile_dst // LIN_MACRO):
                fch = mmpool.tile([128, LIN_MACRO * 128], f32, tag="fch")
                nc.sync.dma_start(
                    out=fch[:], in_=fto[:, m * LIN_MACRO * 128:(m + 1) * LIN_MACRO * 128])
                nrm = mmpool.tile([1, LIN_MACRO * 128], f32, tag="nrm")
                nc.sync.dma_start(
                    out=nrm[:], in_=normr[:, m * LIN_MACRO * 128:(m + 1) * LIN_MACRO * 128])
                gsb = gpool.tile([128, LIN_MACRO, 128], f16, tag="gsb")
                ps = psmm.tile([128, 8, 128], f32, tag="psm")
                for j in range(LIN_MACRO):
                    nc.tensor.matmul(ps[:, j, :], fch[:, j * 128:(j + 1) * 128],
                                     wt_sb[:], start=True, stop=False)
                    nc.tensor.matmul(ps[:, j, :], nrm[:, j * 128:(j + 1) * 128],
                                     bias_sb[:], start=False, stop=True)
                if m % 2 == 0:
                    nc.vector.tensor_copy(
                        gsb[:].rearrange("p a d -> p (a d)"),
                        ps[:, 0:LIN_MACRO, :].rearrange("p a d -> p (a d)"))
                else:
                    nc.scalar.activation(
                        gsb[:].rearrange("p a d -> p (a d)"),
                        ps[:, 0:LIN_MACRO, :].rearrange("p a d -> p (a d)"),
                        mybir.ActivationFunctionType.Copy)
                nc.sync.dma_start(
                    out=gov[m * LIN_MACRO:(m + 1) * LIN_MACRO].rearrange("a p d -> p a d"),
                    in_=gsb[:])
                # chunked table replication: AG slice q as soon as its rows
                # are written (3136 = 3.5 macros; boundaries after macros
                # 3, 6, 10, 13)
                if PHASE != "lin" and m in (3, 6, 10, 13):
                    q = (3, 6, 10, 13).index(m)
                    qs = DSHARD // NSUPER
                    nc.gpsimd.collective_compute(
                        "AllGather", mybir.AluOpType.bypass,
                        [list(range(N_CORES))],
                        [g_own[q * qs:(q + 1) * qs, :]],
                        [g_full[q * SUPER:(q + 1) * SUPER, :]])

            # ---- edge phase ----
            border = [c * NSUPER + s for s in range(NSUPER)
                      for c in range(N_CHUNKS)]
            for bkt in (border if PHASE == "full" else []):
                s = bkt % NSUPER
                tg = epool.tile([128, BLK // 128, D], f16, tag="tg")
                off = 0
                for q, gsz in enumerate((GB, GB, GB, gb4)):
                    c0 = bkt * (BLK // 16) + off // 16
                    nc.gpsimd.dma_gather(
                        tg[:, off // 128:(off + gsz) // 128, :],
                        g_full[s * SUPER:(s + 1) * SUPER, :],
                        gi_sb[:, c0:c0 + gsz // 16], gsz, gsz, D,
                        queue_num=(q + bkt) % 4)
                    off += gsz
                hb = BLK // 2
                sh2 = 3 * GB + gb4 - hb     # real fill beyond half 1
                for h, hsz in enumerate((hb, sh2)):
                    nc.gpsimd.dma_scatter_add(
                        agg[:], tg[:, h * (hb // 128):h * (hb // 128) + hsz // 128, :],
                        si_sb[:, bkt * (BLK // 16) + h * (hb // 16):
                              bkt * (BLK // 16) + h * (hb // 16) + hsz // 16],
                        hsz, hsz, D, queue_num=(bkt + 2 * h + 1) % 4)

            # ---- fixup: replay over-degree edges parked in scratch rows ----
            for fb in (range(NFIX) if PHASE == "full" else []):
                tf = epool.tile([128, FIXBLK // 128, D], f16, tag="tf")
                cols = slice(fb * (FIXBLK // 16), (fb + 1) * (FIXBLK // 16))
                nc.gpsimd.dma_gather(
                    tf[:], agg[:], fgi_sb[:, cols], FIXBLK, FIXBLK, D)
                nc.gpsimd.dma_scatter_add(
                    agg[:], tf[:], fsi_sb[:, cols], FIXBLK, FIXBLK, D)

            # ---- epilogue: out = norm * relu(agg) ----
            for m in (range(ntile_dst // FIN_MACRO) if PHASE == "full" else []):
                asb = fpool.tile([128, FIN_MACRO, 128], f16, tag="asb")
                nc.sync.dma_start(
                    out=asb[:],
                    in_=aggv[m * FIN_MACRO:(m + 1) * FIN_MACRO].rearrange("a p d -> p a d"))
                rsb = fpool.tile([128, FIN_MACRO, 128], f32, tag="rsb")
                nc.scalar.activation(
                    rsb[:].rearrange("p a d -> p (a d)"),
                    asb[:].rearrange("p a d -> p (a d)"),
                    mybir.ActivationFunctionType.Relu)
                nb = bass.AP(ndst_sb.tensor,
                             ndst_sb.offset + m * FIN_MACRO,
                             [[ntile_dst, 128], [1, FIN_MACRO], [0, 128]])
                nc.vector.tensor_tensor(rsb[:], rsb[:], nb, mybir.AluOpType.mult)
                nc.sync.dma_start(
                    out=outv[m * FIN_MACRO:(m + 1) * FIN_MACRO].rearrange("a p d -> p a d"),
                    in_=rsb[:])

    if PHASE != "full":
        # variants still must write the output tensor
        with TileContext(nc) as tc2:
            with tc2.tile_pool(name="dummy", bufs=1) as dpool:
                zz = dpool.tile([128, 10 * 128], f32)
                nc.vector.memset(zz[:], 0.0)
                k = 0
                while k < ntile_dst:
                    gsz = min(10, ntile_dst - k)
                    nc.sync.dma_start(
                        out=outv[k:k + gsz].rearrange("a p d -> p a d"),
                        in_=zz[:, :gsz * 128].rearrange("p (a d) -> p a d", a=gsz))
                    k += gsz
    nc.compile()
    _nc_cache[key] = nc
    return nc


# ---------------------------------------------------------------- host pack
def _wrap16(stream: np.ndarray) -> np.ndarray:
    """idx i at [i%16, i//16], replicated x8 across partition groups."""
    a = stream.astype(np.int16).reshape(-1, 16).T
    return np.tile(a, (8, 1))


def _pack_core_edges(src_c: np.ndarray, dst_l: np.ndarray):
    """Build per-core index streams for the scatter-based edge phase.

    Returns (gidx_stream [NBLK*BLK], sidx_stream [NBLK*BLK],
             fgidx_stream, fsidx_stream)."""
    g_stream = np.zeros(NBLK * BLK, dtype=np.int64)
    s_stream = TRASH + (np.arange(NBLK * BLK) % 128)
    fg_stream = np.zeros(NFIX * FIXBLK, dtype=np.int64)
    fs_stream = TRASH + (np.arange(NFIX * FIXBLK) % 128)

    excess = []  # (dst_local, scratch_row)
    n_scratch = 0

    qc = src_c // DSHARD
    qr = src_c % DSHARD
    sg = qr // (DSHARD // NSUPER)
    sl = qc * (DSHARD // NSUPER) + qr % (DSHARD // NSUPER)
    for s in range(NSUPER):
        m = sg == s
        sls, dls = sl[m], dst_l[m]
        order = np.argsort(dls, kind="stable")
        sls, dls = sls[order], dls[order]
        n = len(dls)
        if n == 0:
            continue
        chunk = np.arange(n) % N_CHUNKS
        first = np.searchsorted(dls, dls, side="left")
        occ = np.arange(n) - first
        ok = occ < N_CHUNKS

        sc_rows = []
        for edl in dls[~ok].tolist():
            assert n_scratch < 128, "scratch overflow"
            excess.append((edl, SCRATCH0 + n_scratch))
            sc_rows.append(SCRATCH0 + n_scratch)
            n_scratch += 1
        dls = dls.copy()
        if sc_rows:
            dls[~ok] = np.asarray(sc_rows, dtype=np.int64)

        corder = np.argsort(chunk, kind="stable")
        sls, dls, chunk = sls[corder], dls[corder], chunk[corder]
        counts = np.bincount(chunk, minlength=N_CHUNKS)
        _pack_core_edges.maxfill = max(
            getattr(_pack_core_edges, "maxfill", 0), int(counts.max()))
        offs = np.concatenate([[0], np.cumsum(counts)])
        for c in range(N_CHUNKS):
            nb = counts[c]
            assert nb <= BLK, f"block overflow {nb} > {BLK}"
            base = (c * NSUPER + s) * BLK
            g_stream[base:base + nb] = sls[offs[c]:offs[c + 1]]
            s_stream[base:base + nb] = dls[offs[c]:offs[c + 1]]

    fill = [0] * NFIX
    fsets = [set() for _ in range(NFIX)]
    for edl, srow in excess:
        for fb in range(NFIX):
            if fill[fb] < FIXBLK and edl not in fsets[fb]:
                fg_stream[fb * FIXBLK + fill[fb]] = srow
                fs_stream[fb * FIXBLK + fill[fb]] = edl
                fsets[fb].add(edl)
                fill[fb] += 1
                break
        else:
            raise RuntimeError("fixup overflow: increase NFIX")
    return g_stream, s_stream, fg_stream, fs_stream


def pack_inputs(features, norm, W, b, src, dst):
    features = np.asarray(features, dtype=np.float32)
    norm = np.asarray(norm, dtype=np.float32).reshape(-1)
    W = np.asarray(W, dtype=np.float32)
    b = np.asarray(b, dtype=np.float32)
    src = np.asarray(src).astype(np.int64)
    dst = np.asarray(dst).astype(np.int64)
    n = features.shape[0]

    xp = features * norm[:, None]          # fold norm[src] into X
    wt = np.ascontiguousarray(W.T)
    biasr = b.reshape(1, 128).astype(np.float32)
    norm_pad = np.zeros(NPAD, dtype=np.float32)
    norm_pad[:n] = norm

    shared = {"wt": wt, "biasr": biasr}

    owner = dst // DSHARD
    in_maps = []
    for c in range(N_CORES):
        m = owner == c
        gs, ss, fgs, fss = _pack_core_edges(src[m], dst[m] - c * DSHARD)
        lo = c * DSHARD
        hi = min(lo + DSHARD, n)
        fto = np.zeros((128, DSHARD), dtype=np.float32)
        if hi > lo:
            fto[:, :hi - lo] = xp[lo:hi].T
        normr = norm_pad[lo:lo + DSHARD].reshape(1, DSHARD)
        normt_dst = np.ascontiguousarray(norm_pad[lo:lo + DSHARD].reshape(-1, 128).T)
        in_maps.append(dict(shared,
                            fto=fto,
                            normr=np.ascontiguousarray(normr),
                            normt_dst=normt_dst,
                            gidx=_wrap16(gs),
                            sidx=_wrap16(ss),
                            fgidx=_wrap16(fgs),
                            fsidx=_wrap16(fss)))
    # trim the 4th gather call of every block to the data's real need
    # (padding slots beyond it scatter stale-but-finite data to trash rows)
    maxfill = _pack_core_edges.maxfill
    gb4 = min(GB, max(128, int(-(-(maxfill + 128 - 3 * GB) // 128)) * 128))
    return in_maps, gb4


def kernel(**inputs) -> np.ndarray:
    in_maps, gb4 = pack_inputs(inputs["features"], inputs["norm"], inputs["W"],
                               inputs["b"], inputs["src"], inputs["dst"])
    nc = build_nc(gb4)
    res = run_bass_kernel_spmd(nc, in_maps, core_ids=list(range(N_CORES)))
    n = np.asarray(inputs["features"]).shape[0]
    out = np.empty((n, D), dtype=np.float32)
    for c in range(N_CORES):
        lo = c * DSHARD
        hi = min(lo + DSHARD, n)
        if hi > lo:
            out[lo:hi] = res.results[c]["out"][:hi - lo]
    return out

